# revision 9
# baseline (speedup 1.0000x reference)
"""Trainium2 Bass kernel for a bidirectional cross-attention layer (v4).

Per batch sample (one NeuronCore each, 8 samples / 8 cores):
    e  = seq_1 @ seq_2^T                     [L, L]
    P  = exp(e)            (no max-subtraction: |e| <~ 70 << fp32 overflow)
    seq_1_hat = diag(1/rowsum(P)) @ P   @ seq_2
    seq_2_hat = diag(1/colsum(P)) @ P^T @ seq_1

v4 (vs v3): single fused phase keeping the PE dense (so it holds the
2.4 GHz p-state): o1 chunk GEMMs interleave into the block stream as
their P^T stripes land.  One DVE scalar_tensor_tensor per block both
accumulates a running [128, L] column-sum tile (run += P_b) and emits
cumulative row sums via accum_out (true rowsums recovered by cheap
differencing).  colsum finishes with a single GpSimd partition reduce
in the tail.  o2 normalization runs on ACT (idle after the exps) via
activation-with-scale.  DMA triggers spread over SP/ACT/Pool queues.
"""

import os

os.environ.setdefault("MYCRO_LOCAL_CACHE", "1")

import numpy as np

import concourse.mybir as mybir
from concourse import bacc
from concourse.bass_utils import run_bass_kernel_spmd
from concourse.tile import TileContext

B, L, D = 8, 2048, 128
NBLK = L // 128  # 16 blocks of 128
NCH = L // 512   # 4 chunks of 512

F32 = mybir.dt.float32
BF16 = mybir.dt.bfloat16
AF = mybir.ActivationFunctionType
ALU = mybir.AluOpType
AX = mybir.AxisListType


def _build():
    nc = bacc.Bacc(
        "TRN2", target_bir_lowering=False, debug=False, enable_asserts=False
    )
    s1 = nc.dram_tensor("seq_1", [L, D], F32, kind="ExternalInput").ap()
    s2 = nc.dram_tensor("seq_2", [L, D], F32, kind="ExternalInput").ap()
    o1 = nc.dram_tensor("out1", [L, D], F32, kind="ExternalOutput").ap()
    o2 = nc.dram_tensor("out2", [L, D], F32, kind="ExternalOutput").ap()

    with TileContext(nc) as tc:
        with (
            tc.tile_pool(name="big", bufs=1) as big,
            tc.tile_pool(name="pbp", bufs=4) as pbp,
            tc.tile_pool(name="scrp", bufs=2) as scrp,
            tc.tile_pool(name="outp", bufs=2) as outp,
            tc.tile_pool(name="etp", bufs=3, space="PSUM") as etp,
            tc.tile_pool(name="acc2p", bufs=1, space="PSUM") as acc2p,
            tc.tile_pool(name="acc1p", bufs=1, space="PSUM") as acc1p,
        ):
            # ---- persistent SBUF tensors -------------------------------
            s1f = big.tile([128, L], F32, tag="s1f")    # [i%128, (blk,d)]
            s2f = big.tile([128, L], F32, tag="s2f")
            s1b = big.tile([128, L], BF16, tag="s1b")   # bf16 casts
            s2b = big.tile([128, L], BF16, tag="s2b")
            s1t = big.tile([128, NBLK, 128], BF16, tag="s1t")  # [d, blk, i%128]
            s2t = big.tile([128, NBLK, 128], BF16, tag="s2t")
            ptp = big.tile([128, NBLK, L], BF16, tag="ptp")  # [j%128, jblk, i]
            runA = big.tile([128, L], F32, tag="runA")  # colsum partial (ping)
            runB = big.tile([128, L], F32, tag="runB")  # colsum partial (pong)
            rsum = big.tile([128, NBLK], F32, tag="rsum")  # per-block rowsums
            rrow = big.tile([128, NBLK], F32, tag="rrow")
            csumF = big.tile([1, L], F32, tag="csumF")
            rcolraw = big.tile([128, NBLK], F32, tag="rcolraw")
            rcol = big.tile([128, NBLK], F32, tag="rcol")

            run = [runA, runB]

            # ---- preload -----------------------------------------------
            # s2 loads on SP, s1 loads on Pool; casts on DVE; s2t XBARs on
            # ACT (idle pre-exp); s1t XBARs on SP (before the P XBARs).
            for g in range(4):
                sl = slice(g * 512, (g + 1) * 512)
                nc.sync.dma_start(
                    s2f[:, sl].rearrange("p (blk d) -> p blk d", blk=4),
                    s2[sl, :].rearrange("(blk p) d -> p blk d", blk=4),
                )
            for g in range(4):
                sl = slice(g * 512, (g + 1) * 512)
                nc.gpsimd.dma_start(
                    s1f[:, sl].rearrange("p (blk d) -> p blk d", blk=4),
                    s1[sl, :].rearrange("(blk p) d -> p blk d", blk=4),
                )
            for g in range(4):
                sl = slice(g * 512, (g + 1) * 512)
                nc.vector.tensor_copy(s2b[:, sl], s2f[:, sl])
                nc.scalar.dma_start(
                    s2t[:, 4 * g:4 * g + 4, :], s2b[:, sl], transpose=True
                )
                nc.vector.tensor_copy(s1b[:, sl], s1f[:, sl])
                nc.sync.dma_start(
                    s1t[:, 4 * g:4 * g + 4, :], s1b[:, sl], transpose=True
                )

            acc2 = acc2p.tile([128, L], F32, tag="acc2")

            def o1_chunk(k):
                """o1^T[d, i-chunk k] = sum_c s2b[:,c]^T @ ptp[:, c, ksl];
                then normalize rows by 1/rowsum and store."""
                ksl = slice(k * 512, (k + 1) * 512)
                acc1 = acc1p.tile([128, 512], F32, tag="acc1")
                for c in range(NBLK):
                    nc.tensor.matmul(
                        acc1,
                        lhsT=s2b[:, c * 128:(c + 1) * 128],
                        rhs=ptp[:, c, ksl],
                        start=(c == 0), stop=(c == NBLK - 1),
                    )
                nc.vector.reciprocal(rrow[:, 4 * k:4 * k + 4],
                                     rsum[:, 4 * k:4 * k + 4])
                bb1 = outp.tile([128, 512], BF16, tag="bb1")
                nc.vector.tensor_copy(bb1, acc1)
                tb1 = outp.tile([128, 4, 128], BF16, tag="tb1")
                nc.sync.dma_start(tb1, bb1, transpose=True)
                of1 = outp.tile([128, 512], F32, tag="of1")
                for c2 in range(4):
                    blk = 4 * k + c2
                    nc.vector.tensor_scalar_mul(
                        of1[:, c2 * 128:(c2 + 1) * 128],
                        tb1[:, c2, :], rrow[:, blk:blk + 1],
                    )
                nc.gpsimd.dma_start(
                    o1[ksl, :].rearrange("(c p) d -> p c d", c=4),
                    of1.rearrange("p (c d) -> p c d", c=4),
                )

            # ---- fused main phase --------------------------------------
            for b in range(NBLK):
                bsl = slice(b * 128, (b + 1) * 128)
                pb = pbp.tile([128, L], BF16, tag="pb")
                for q in range(4):
                    qsl = slice(q * 512, (q + 1) * 512)
                    et = etp.tile([128, 512], F32, tag="et")
                    nc.tensor.matmul(
                        et, lhsT=s1t[:, b, :], rhs=s2t[:, 4 * q:4 * q + 4, :],
                        start=True, stop=True,
                    )
                    nc.scalar.activation(pb[:, qsl], et, AF.Exp)
                # rowsum for this block (DVE 4x: all-bf16 contiguous SBUF)
                scr = scrp.tile([128, L], BF16, tag="scr")
                nc.vector.tensor_scalar(
                    scr, pb, 1.0, 0.0, op0=ALU.mult, op1=ALU.add,
                    accum_out=rsum[:, b:b + 1],
                )
                # colsum running accumulation (ping-pong fp32 tiles)
                if b == 0:
                    nc.vector.tensor_scalar(
                        run[0], pb, 1.0, 0.0, op0=ALU.mult, op1=ALU.add,
                    )
                else:
                    nc.vector.scalar_tensor_tensor(
                        run[b % 2], pb, 1.0, run[(b + 1) % 2],
                        op0=ALU.mult, op1=ALU.add,
                    )
                for q in range(4):
                    qsl = slice(q * 512, (q + 1) * 512)
                    nc.tensor.matmul(
                        acc2[:, qsl],
                        lhsT=s1b[:, bsl],
                        rhs=pb[:, qsl],
                        start=(b == 0), stop=(b == NBLK - 1),
                    )
                nc.sync.dma_start(ptp[:, :, bsl], pb, transpose=True)
                if b in (4, 8, 12):
                    o1_chunk(b // 4 - 1)

            # ---- tail --------------------------------------------------
            o1_chunk(3)
            # colsum finalize: partition-reduce the running sums, writing
            # element j=c*128+p to address p*16+c so the redistribute DMA
            # below is a plain contiguous-split copy
            nc.gpsimd.tensor_reduce(
                csumF.rearrange("q (p c) -> q c p", p=128),
                run[(NBLK - 1) % 2], axis=AX.C, op=ALU.add,
            )
            nc.sync.dma_start(rcolraw, csumF)
            nc.vector.reciprocal(rcol, rcolraw)
            for ch in range(NCH):
                jsl = slice(ch * 512, (ch + 1) * 512)
                bb2 = outp.tile([128, 512], BF16, tag="bb2")
                nc.vector.tensor_copy(bb2, acc2[:, jsl])
                tb2 = outp.tile([128, 4, 128], BF16, tag="tb2")
                nc.sync.dma_start(tb2, bb2, transpose=True)
                of2 = outp.tile([128, 512], F32, tag="of2")
                for c2 in range(4):
                    blk = 4 * ch + c2
                    nc.scalar.activation(
                        of2[:, c2 * 128:(c2 + 1) * 128], tb2[:, c2, :],
                        AF.Copy, scale=rcol[:, blk:blk + 1],
                    )
                nc.gpsimd.dma_start(
                    o2[jsl, :].rearrange("(c p) d -> p c d", c=4),
                    of2.rearrange("p (c d) -> p c d", c=4),
                )

    nc.compile()
    return nc


_nc_cache = None


def _run(seq_1, seq_2, trace=False):
    global _nc_cache
    if _nc_cache is None:
        _nc_cache = _build()
    nc = _nc_cache
    seq_1 = np.ascontiguousarray(np.asarray(seq_1, dtype=np.float32))
    seq_2 = np.ascontiguousarray(np.asarray(seq_2, dtype=np.float32))
    in_maps = [{"seq_1": seq_1[b], "seq_2": seq_2[b]} for b in range(B)]
    res = run_bass_kernel_spmd(nc, in_maps, core_ids=list(range(B)), trace=trace)
    out1 = np.stack([res.results[b]["out1"] for b in range(B)])
    out2 = np.stack([res.results[b]["out2"] for b in range(B)])
    return (out1, out2), res


def kernel(seq_1, seq_2):
    return _run(seq_1, seq_2)[0]


# revision 13
# speedup vs baseline: 2.6832x; 2.6832x over previous
"""Trainium2 Bass kernel for a bidirectional cross-attention layer (v4).

Per batch sample (one NeuronCore each, 8 samples / 8 cores):
    e  = seq_1 @ seq_2^T                     [L, L]
    P  = exp(e)            (no max-subtraction: |e| <~ 70 << fp32 overflow)
    seq_1_hat = diag(1/rowsum(P)) @ P   @ seq_2
    seq_2_hat = diag(1/colsum(P)) @ P^T @ seq_1

v4 (vs v3): single fused phase keeping the PE dense (so it holds the
2.4 GHz p-state): o1 chunk GEMMs interleave into the block stream as
their P^T stripes land.  One DVE scalar_tensor_tensor per block both
accumulates a running [128, L] column-sum tile (run += P_b) and emits
cumulative row sums via accum_out (true rowsums recovered by cheap
differencing).  colsum finishes with a single GpSimd partition reduce
in the tail.  o2 normalization runs on ACT (idle after the exps) via
activation-with-scale.  DMA triggers spread over SP/ACT/Pool queues.
"""

import os

os.environ.setdefault("MYCRO_LOCAL_CACHE", "1")

import numpy as np

import concourse.mybir as mybir
from concourse import bacc
from concourse.bass_utils import run_bass_kernel_spmd
from concourse.tile import TileContext

B, L, D = 8, 2048, 128
NBLK = L // 128  # 16 blocks of 128
NCH = L // 512   # 4 chunks of 512

F32 = mybir.dt.float32
BF16 = mybir.dt.bfloat16
AF = mybir.ActivationFunctionType
ALU = mybir.AluOpType
AX = mybir.AxisListType


def _build():
    nc = bacc.Bacc(
        "TRN2", target_bir_lowering=False, debug=False, enable_asserts=False
    )
    s1 = nc.dram_tensor("seq_1", [L, D], F32, kind="ExternalInput").ap()
    s2 = nc.dram_tensor("seq_2", [L, D], F32, kind="ExternalInput").ap()
    o1 = nc.dram_tensor("out1", [L, D], F32, kind="ExternalOutput").ap()
    o2 = nc.dram_tensor("out2", [L, D], F32, kind="ExternalOutput").ap()

    with TileContext(nc) as tc:
        with (
            tc.tile_pool(name="big", bufs=1) as big,
            tc.tile_pool(name="pbp", bufs=4) as pbp,
            tc.tile_pool(name="scrp", bufs=2) as scrp,
            tc.tile_pool(name="outp", bufs=2) as outp,
            tc.tile_pool(name="etp", bufs=3, space="PSUM") as etp,
            tc.tile_pool(name="acc2p", bufs=1, space="PSUM") as acc2p,
            tc.tile_pool(name="acc1p", bufs=1, space="PSUM") as acc1p,
        ):
            # ---- persistent SBUF tensors -------------------------------
            s1f = big.tile([128, L], F32, tag="s1f")    # [i%128, (blk,d)]
            s2f = big.tile([128, L], F32, tag="s2f")
            s1b = big.tile([128, L], BF16, tag="s1b")   # bf16 casts
            s2b = big.tile([128, L], BF16, tag="s2b")
            s1t = big.tile([128, NBLK, 128], BF16, tag="s1t")  # [d, blk, i%128]
            s2t = big.tile([128, NBLK, 128], BF16, tag="s2t")
            ptp = big.tile([128, NBLK, L], BF16, tag="ptp")  # [j%128, jblk, i]
            runA = big.tile([128, L], BF16, tag="runA")  # colsum partial (ping)
            runB = big.tile([128, L], BF16, tag="runB")  # colsum partial (pong)
            ones = big.tile([128, 1], BF16, tag="ones")
            rsum = big.tile([128, NBLK], F32, tag="rsum")  # per-block rowsums
            rrow = big.tile([128, NBLK], F32, tag="rrow")
            csumF = big.tile([1, L], F32, tag="csumF")
            rcolraw = big.tile([128, NBLK], F32, tag="rcolraw")
            rcol = big.tile([128, NBLK], F32, tag="rcol")

            run = [runA, runB]
            # colsum accumulation split point: DVE handles [:CS], GpSimd [CS:]
            CS = 768
            nc.gpsimd.memset(ones, 1.0)

            # ---- preload -----------------------------------------------
            # s2 loads on SP, s1 loads on Pool; casts on DVE; s2t XBARs on
            # ACT (idle pre-exp); s1t XBARs on SP (before the P XBARs).
            for g in range(4):
                sl = slice(g * 512, (g + 1) * 512)
                nc.sync.dma_start(
                    s2f[:, sl].rearrange("p (blk d) -> p blk d", blk=4),
                    s2[sl, :].rearrange("(blk p) d -> p blk d", blk=4),
                )
            for g in range(4):
                sl = slice(g * 512, (g + 1) * 512)
                nc.gpsimd.dma_start(
                    s1f[:, sl].rearrange("p (blk d) -> p blk d", blk=4),
                    s1[sl, :].rearrange("(blk p) d -> p blk d", blk=4),
                )
            for g in range(4):
                sl = slice(g * 512, (g + 1) * 512)
                nc.vector.tensor_copy(s2b[:, sl], s2f[:, sl])
                nc.scalar.dma_start(
                    s2t[:, 4 * g:4 * g + 4, :], s2b[:, sl], transpose=True
                )
                nc.vector.tensor_copy(s1b[:, sl], s1f[:, sl])
                nc.sync.dma_start(
                    s1t[:, 4 * g:4 * g + 4, :], s1b[:, sl], transpose=True
                )

            acc2 = acc2p.tile([128, L], F32, tag="acc2")

            def o1_chunk(k):
                """o1^T[d, i-chunk k] = sum_c s2b[:,c]^T @ ptp[:, c, ksl];
                then normalize rows by 1/rowsum and store."""
                ksl = slice(k * 512, (k + 1) * 512)
                acc1 = acc1p.tile([128, 512], F32, tag="acc1")
                for c in range(NBLK):
                    nc.tensor.matmul(
                        acc1,
                        lhsT=s2b[:, c * 128:(c + 1) * 128],
                        rhs=ptp[:, c, ksl],
                        start=(c == 0), stop=(c == NBLK - 1),
                    )
                nc.vector.reciprocal(rrow[:, 4 * k:4 * k + 4],
                                     rsum[:, 4 * k:4 * k + 4])
                bb1 = outp.tile([128, 512], BF16, tag="bb1")
                nc.vector.tensor_copy(bb1, acc1)
                tb1 = outp.tile([128, 4, 128], BF16, tag="tb1")
                nc.sync.dma_start(tb1, bb1, transpose=True)
                of1 = outp.tile([128, 512], F32, tag="of1")
                for c2 in range(4):
                    blk = 4 * k + c2
                    nc.vector.tensor_scalar_mul(
                        of1[:, c2 * 128:(c2 + 1) * 128],
                        tb1[:, c2, :], rrow[:, blk:blk + 1],
                    )
                nc.gpsimd.dma_start(
                    o1[ksl, :].rearrange("(c p) d -> p c d", c=4),
                    of1.rearrange("p (c d) -> p c d", c=4),
                )

            # ---- fused main phase --------------------------------------
            for b in range(NBLK):
                bsl = slice(b * 128, (b + 1) * 128)
                pb = pbp.tile([128, L], BF16, tag="pb")
                for q in range(4):
                    qsl = slice(q * 512, (q + 1) * 512)
                    et = etp.tile([128, 512], F32, tag="et")
                    nc.tensor.matmul(
                        et, lhsT=s1t[:, b, :], rhs=s2t[:, 4 * q:4 * q + 4, :],
                        start=True, stop=True,
                    )
                    nc.scalar.activation(pb[:, qsl], et, AF.Exp)
                # rowsum for this block (DVE 4x: all-bf16 contiguous SBUF)
                scr = scrp.tile([128, L], BF16, tag="scr")
                nc.vector.tensor_scalar(
                    scr, pb, 1.0, 0.0, op0=ALU.mult, op1=ALU.add,
                    accum_out=rsum[:, b:b + 1],
                )
                # colsum running accumulation (bf16 ping-pong, split DVE/GpSimd)
                if b == 0:
                    nc.vector.tensor_copy(run[0][:, :CS], pb[:, :CS])
                    nc.gpsimd.tensor_copy(run[0][:, CS:], pb[:, CS:])
                else:
                    nc.vector.scalar_tensor_tensor(
                        run[b % 2][:, :CS], pb[:, :CS], 1.0,
                        run[(b + 1) % 2][:, :CS],
                        op0=ALU.mult, op1=ALU.add,
                    )
                    nc.gpsimd.tensor_tensor(
                        run[b % 2][:, CS:], pb[:, CS:],
                        run[(b + 1) % 2][:, CS:], op=ALU.add,
                    )
                for q in range(4):
                    qsl = slice(q * 512, (q + 1) * 512)
                    nc.tensor.matmul(
                        acc2[:, qsl],
                        lhsT=s1b[:, bsl],
                        rhs=pb[:, qsl],
                        start=(b == 0), stop=(b == NBLK - 1),
                    )
                nc.sync.dma_start(ptp[:, :, bsl], pb, transpose=True)
                if b in (4, 8, 12):
                    o1_chunk(b // 4 - 1)

            # ---- tail --------------------------------------------------
            o1_chunk(3)
            # colsum finalize: PE ones-matvec folds the 128 partitions of
            # the run tile; ACT copies each [1,512] quarter into csumF with
            # a permuted AP (element j=c*128+p lands at address p*16+c) so
            # the redistribute DMA below is a plain contiguous-split copy
            rfin = run[(NBLK - 1) % 2]
            csumFv = csumF.rearrange("q (p c) -> q c p", p=128)
            for q in range(4):
                mv = acc1p.tile([128, 512], F32, tag="acc1")
                nc.tensor.matmul(
                    mv[0:1, :], lhsT=ones, rhs=rfin[:, q * 512:(q + 1) * 512],
                    start=True, stop=True,
                )
                nc.scalar.activation(
                    csumFv[:, 4 * q:4 * q + 4, :], mv[0:1, :], AF.Copy,
                )
            nc.sync.dma_start(rcolraw, csumF)
            nc.vector.reciprocal(rcol, rcolraw)
            for ch in range(NCH):
                jsl = slice(ch * 512, (ch + 1) * 512)
                bb2 = outp.tile([128, 512], BF16, tag="bb2")
                nc.vector.tensor_copy(bb2, acc2[:, jsl])
                tb2 = outp.tile([128, 4, 128], BF16, tag="tb2")
                nc.sync.dma_start(tb2, bb2, transpose=True)
                of2 = outp.tile([128, 512], F32, tag="of2")
                for c2 in range(4):
                    blk = 4 * ch + c2
                    nc.scalar.activation(
                        of2[:, c2 * 128:(c2 + 1) * 128], tb2[:, c2, :],
                        AF.Copy, scale=rcol[:, blk:blk + 1],
                    )
                nc.gpsimd.dma_start(
                    o2[jsl, :].rearrange("(c p) d -> p c d", c=4),
                    of2.rearrange("p (c d) -> p c d", c=4),
                )

    nc.compile()
    return nc


_nc_cache = None


def _run(seq_1, seq_2, trace=False):
    global _nc_cache
    if _nc_cache is None:
        _nc_cache = _build()
    nc = _nc_cache
    seq_1 = np.ascontiguousarray(np.asarray(seq_1, dtype=np.float32))
    seq_2 = np.ascontiguousarray(np.asarray(seq_2, dtype=np.float32))
    in_maps = [{"seq_1": seq_1[b], "seq_2": seq_2[b]} for b in range(B)]
    res = run_bass_kernel_spmd(nc, in_maps, core_ids=list(range(B)), trace=trace)
    out1 = np.stack([res.results[b]["out1"] for b in range(B)])
    out2 = np.stack([res.results[b]["out2"] for b in range(B)])
    return (out1, out2), res


def kernel(seq_1, seq_2):
    return _run(seq_1, seq_2)[0]


# revision 19
# speedup vs baseline: 2.8097x; 1.0471x over previous
"""Trainium2 Bass kernel for a bidirectional cross-attention layer (v6).

Per batch sample (one NeuronCore each, 8 samples / 8 cores):
    e  = seq_1 @ seq_2^T                     [L, L]
    P  = exp(e)            (no max-subtraction: |e| <~ 70 << fp32 overflow)
    seq_1_hat = diag(1/rowsum(P)) @ P   @ seq_2
    seq_2_hat = diag(1/colsum(P)) @ P^T @ seq_1

v6: colsum rides the PE as per-block ones-matvecs accumulating into a
single PSUM bank (one [1,512] accumulator per j-quarter, parked at
partitions 0/32/64/96 of the same bank via 32-aligned tile positions).
That keeps the PE dense (scores + o2 + colsum ~2.6us/block, holding
the 2.4 GHz p-state) while ACT runs the exps and DVE only does the
rowsum accumulation.  o1 runs as a phase B overlapped with both
output epilogues; o2 normalization uses ACT activation-with-scale.
"""

import os

os.environ.setdefault("MYCRO_LOCAL_CACHE", "1")

import numpy as np

import concourse.mybir as mybir
from concourse import bacc
from concourse.bass_utils import run_bass_kernel_spmd
from concourse.tile import TileContext

B, L, D = 8, 2048, 128
NBLK = L // 128  # 16 blocks of 128
NCH = L // 512   # 4 chunks of 512

F32 = mybir.dt.float32
BF16 = mybir.dt.bfloat16
AF = mybir.ActivationFunctionType
ALU = mybir.AluOpType
AX = mybir.AxisListType


def _build():
    nc = bacc.Bacc(
        "TRN2", target_bir_lowering=False, debug=False, enable_asserts=False
    )
    s1 = nc.dram_tensor("seq_1", [L, D], F32, kind="ExternalInput").ap()
    s2 = nc.dram_tensor("seq_2", [L, D], F32, kind="ExternalInput").ap()
    o1 = nc.dram_tensor("out1", [L, D], F32, kind="ExternalOutput").ap()
    o2 = nc.dram_tensor("out2", [L, D], F32, kind="ExternalOutput").ap()

    with TileContext(nc) as tc:
        with (
            tc.tile_pool(name="big", bufs=1) as big,
            tc.tile_pool(name="pbp", bufs=4) as pbp,
            tc.tile_pool(name="scrp", bufs=2) as scrp,
            tc.tile_pool(name="outp", bufs=2) as outp,
            tc.tile_pool(name="etp", bufs=2, space="PSUM") as etp,
            tc.tile_pool(name="acc2p", bufs=1, space="PSUM") as acc2p,
            tc.tile_pool(name="mvp", bufs=1, space="PSUM") as mvp,
            tc.tile_pool(name="acc1p", bufs=1, space="PSUM") as acc1p,
        ):
            # ---- persistent SBUF tensors -------------------------------
            s1f = big.tile([128, L], F32, tag="s1f")    # [i%128, (blk,d)]
            s2f = big.tile([128, L], F32, tag="s2f")
            s1b = big.tile([128, L], BF16, tag="s1b")   # bf16 casts
            s2b = big.tile([128, L], BF16, tag="s2b")
            s1t = big.tile([128, NBLK, 128], BF16, tag="s1t")  # [d, blk, i%128]
            s2t = big.tile([128, NBLK, 128], BF16, tag="s2t")
            ptp = big.tile([128, NBLK, L], BF16, tag="ptp")  # [j%128, jblk, i]
            ones = big.tile([128, 1], BF16, tag="ones")
            rsum = big.tile([128, NBLK], F32, tag="rsum")  # per-block rowsums
            rrow = big.tile([128, NBLK], F32, tag="rrow")
            csumF = big.tile([1, L], F32, tag="csumF")
            rcolraw = big.tile([128, NBLK], F32, tag="rcolraw")
            rcol = big.tile([128, NBLK], F32, tag="rcol")

            nc.gpsimd.memset(ones, 1.0)

            # ---- preload -----------------------------------------------
            # s2 via SP loads + DVE casts (earliest-needed path); s1 via
            # GpSimd SWDGE casting DMAs straight to bf16.  s2t XBARs on
            # ACT (idle pre-exp); s1t XBARs on SP.
            for g in range(4):
                sl = slice(g * 512, (g + 1) * 512)
                nc.sync.dma_start(
                    s2f[:, sl].rearrange("p (blk d) -> p blk d", blk=4),
                    s2[sl, :].rearrange("(blk p) d -> p blk d", blk=4),
                )
            for g in range(4):
                sl = slice(g * 512, (g + 1) * 512)
                nc.gpsimd.dma_start(
                    s1f[:, sl].rearrange("p (blk d) -> p blk d", blk=4),
                    s1[sl, :].rearrange("(blk p) d -> p blk d", blk=4),
                )
            for g in range(4):
                sl = slice(g * 512, (g + 1) * 512)
                nc.vector.tensor_copy(s2b[:, sl], s2f[:, sl])
                nc.scalar.dma_start(
                    s2t[:, 4 * g:4 * g + 4, :], s2b[:, sl], transpose=True
                )
                nc.vector.tensor_copy(s1b[:, sl], s1f[:, sl])
                nc.sync.dma_start(
                    s1t[:, 4 * g:4 * g + 4, :], s1b[:, sl], transpose=True
                )

            acc2 = acc2p.tile([128, L], F32, tag="acc2")
            # colsum accumulators: two [1,512] per PSUM bank at partitions
            # {0, 64}; the second bank is acc1's (free until phase B, the
            # pool WAR dependency sequences the handoff)
            mvacc = mvp.tile([128, 512], F32, tag="mvacc")
            mvacc2 = acc1p.tile([128, 512], F32, tag="acc1")
            mvq = [(mvacc, 0), (mvacc, 64), (mvacc2, 0), (mvacc2, 64)]

            # ---- fused main phase --------------------------------------
            for b in range(NBLK):
                bsl = slice(b * 128, (b + 1) * 128)
                pb = pbp.tile([128, L], BF16, tag="pb")
                for q in range(4):
                    qsl = slice(q * 512, (q + 1) * 512)
                    et = etp.tile([128, 512], F32, tag="et")
                    nc.tensor.matmul(
                        et, lhsT=s1t[:, b, :], rhs=s2t[:, 4 * q:4 * q + 4, :],
                        start=True, stop=True,
                    )
                    nc.scalar.activation(pb[:, qsl], et, AF.Exp)
                # colsum partials on PE: ones-matvec per j-quarter into
                # four [1,512] accumulators parked at partitions 32q of
                # one PSUM bank
                for q in range(4):
                    qsl = slice(q * 512, (q + 1) * 512)
                    mt, mp = mvq[q]
                    nc.tensor.matmul(
                        mt[mp:mp + 1, :],
                        lhsT=ones, rhs=pb[:, qsl],
                        start=(b == 0), stop=(b == NBLK - 1),
                    )
                for q in range(4):
                    qsl = slice(q * 512, (q + 1) * 512)
                    nc.tensor.matmul(
                        acc2[:, qsl],
                        lhsT=s1b[:, bsl],
                        rhs=pb[:, qsl],
                        start=(b == 0), stop=(b == NBLK - 1),
                    )
                # rowsum for this block (DVE)
                scr = scrp.tile([128, L], BF16, tag="scr")
                nc.vector.tensor_scalar(
                    scr, pb, 1.0, 0.0, op0=ALU.mult, op1=ALU.add,
                    accum_out=rsum[:, b:b + 1],
                )
                nc.sync.dma_start(ptp[:, :, bsl], pb, transpose=True)

            # ---- phase B: colsum extract, o1 chunks, both epilogues ----
            # colsum: ACT copies each [1,512] quarter into csumF with a
            # permuted AP (element j=c*128+p lands at address p*16+c) so
            # the redistribute DMA below is a plain contiguous-split copy
            csumFv = csumF.rearrange("q (p c) -> q c p", p=128)
            for q in range(4):
                mt, mp = mvq[q]
                nc.scalar.activation(
                    csumFv[:, 4 * q:4 * q + 4, :],
                    mt[mp:mp + 1, :], AF.Copy,
                )
            nc.sync.dma_start(rcolraw, csumF)
            nc.vector.reciprocal(rcol, rcolraw)
            nc.vector.reciprocal(rrow, rsum)

            for k in range(NCH):
                # o1 chunk k: o1^T[d, i-chunk] = sum_c s2b[:,c]^T @ ptp
                ksl = slice(k * 512, (k + 1) * 512)
                acc1 = acc1p.tile([128, 512], F32, tag="acc1")
                for c in range(NBLK):
                    nc.tensor.matmul(
                        acc1,
                        lhsT=s2b[:, c * 128:(c + 1) * 128],
                        rhs=ptp[:, c, ksl],
                        start=(c == 0), stop=(c == NBLK - 1),
                    )
                bb1 = outp.tile([128, 512], BF16, tag="bb1")
                nc.vector.tensor_copy(bb1, acc1)
                tb1 = outp.tile([128, 4, 128], BF16, tag="tb1")
                nc.sync.dma_start(tb1, bb1, transpose=True)
                of1 = outp.tile([128, 512], F32, tag="of1")
                for c2 in range(4):
                    blk = 4 * k + c2
                    nc.vector.tensor_scalar_mul(
                        of1[:, c2 * 128:(c2 + 1) * 128],
                        tb1[:, c2, :], rrow[:, blk:blk + 1],
                    )
                nc.gpsimd.dma_start(
                    o1[ksl, :].rearrange("(c p) d -> p c d", c=4),
                    of1.rearrange("p (c d) -> p c d", c=4),
                )
                # o2 chunk k epilogue (acc2 ready since end of phase A)
                bb2 = outp.tile([128, 512], BF16, tag="bb2")
                nc.vector.tensor_copy(bb2, acc2[:, ksl])
                tb2 = outp.tile([128, 4, 128], BF16, tag="tb2")
                nc.sync.dma_start(tb2, bb2, transpose=True)
                of2 = outp.tile([128, 512], F32, tag="of2")
                for c2 in range(4):
                    blk = 4 * k + c2
                    nc.scalar.activation(
                        of2[:, c2 * 128:(c2 + 1) * 128], tb2[:, c2, :],
                        AF.Copy, scale=rcol[:, blk:blk + 1],
                    )
                nc.gpsimd.dma_start(
                    o2[ksl, :].rearrange("(c p) d -> p c d", c=4),
                    of2.rearrange("p (c d) -> p c d", c=4),
                )

    nc.compile()
    return nc


_nc_cache = None


def _run(seq_1, seq_2, trace=False):
    global _nc_cache
    if _nc_cache is None:
        _nc_cache = _build()
    nc = _nc_cache
    seq_1 = np.ascontiguousarray(np.asarray(seq_1, dtype=np.float32))
    seq_2 = np.ascontiguousarray(np.asarray(seq_2, dtype=np.float32))
    in_maps = [{"seq_1": seq_1[b], "seq_2": seq_2[b]} for b in range(B)]
    res = run_bass_kernel_spmd(nc, in_maps, core_ids=list(range(B)), trace=trace)
    out1 = np.stack([res.results[b]["out1"] for b in range(B)])
    out2 = np.stack([res.results[b]["out2"] for b in range(B)])
    return (out1, out2), res


def kernel(seq_1, seq_2):
    return _run(seq_1, seq_2)[0]
